# revision 12
# baseline (speedup 1.0000x reference)
"""Bass/Trainium2 kernel for nn_CrossEntropyLoss3 (penalized cross-entropy loss).

Reference semantics (B=1048576, W=64):
    p = softmax(predict, axis=-1)
    c = argmax(target, axis=-1)            # target is one-hot float
    alpha = penalty_matrix[c]              # row gather
    loss = -log(p) * target * (exp(1 - p*alpha) - 1)
    out = mean(sum(loss, axis=-1))

target is exactly one-hot (0.0/1.0) and penalty_matrix has a unit diagonal
(the reference builds it with jnp.where(eye, 1.0, pm)), so the per-row loss
collapses to
    loss_i = -log(p_c) * (exp(1 - p_c) - 1),   p_c = exp(x_c) / sum_j exp(x_j)
where exp(x_c) = sum_j target[i,j] * exp(predict[i,j]).

Data-parallel over 8 NeuronCores, B_LOC=131072 rows per core. Per core
(raw bass, explicit semaphores — the walrus codegen on this path allows only
ONE embedded sync-wait per instruction, so all waits are standalone wait_ge):

    per [128 x (T rows x 64)] tile:
        e = exp(x)                      (ACT)
        s[rows] = sum_j e               (DVE grouped reduce, axis=X)
        w = t * e (in place)            (DVE)
        u[rows] = sum_j w               (DVE grouped reduce)  == exp(x_c)
    then on [128, 1024] row vectors:
        a  = ln(u) - ln(s)              # log p_c
        g  = exp(1 - exp(a))            # exp(1 - p_c)
        q  = (g - 1) * a                # == -loss per row
        partial = sum_rows q            -> DRAM [128,1]
    host: out = -sum(partials) / B
"""

import numpy as np

B = 1048576
W = 64
NCORES = 8
B_LOC = B // NCORES  # 131072
P = 128

_CACHE = {}


def _build(b_loc=B_LOC, t_rows=128):
    import concourse.bass as bass
    import concourse.mybir as mybir
    from concourse.bass import compact_to_ranges

    f32 = mybir.dt.float32
    AF = mybir.ActivationFunctionType
    ALU = mybir.AluOpType
    AX = mybir.AxisListType

    rpp = b_loc // P          # rows per partition
    nt = rpp // t_rows        # tiles
    fd = t_rows * W           # free-dim elements per partition per tile

    nc = bass.Bass()
    pred = nc.declare_dram_parameter("predict", [b_loc, W], f32, isOutput=False)
    targ = nc.declare_dram_parameter("target", [b_loc, W], f32, isOutput=False)
    part = nc.declare_dram_parameter("partial", [P, 1], f32, isOutput=True)

    pred_v = pred[:].rearrange("(p n) w -> p n w", p=P)
    targ_v = targ[:].rearrange("(p n) w -> p n w", p=P)

    with (
        nc.sbuf_tensor("x_sb", [P, 2 * fd], f32) as x_sb,
        nc.sbuf_tensor("t_sb", [P, 2 * fd], f32) as t_sb,
        nc.sbuf_tensor("e_sb", [P, 2 * fd], f32) as e_sb,
        nc.sbuf_tensor("s_all", [P, rpp], f32) as s_all,
        nc.sbuf_tensor("z_all", [P, rpp], f32) as z_all,
        nc.sbuf_tensor("g_all", [P, rpp], f32) as g_all,
        nc.sbuf_tensor("out_t", [P, 1], f32) as out_t,
        nc.semaphore("dx0") as dx0,
        nc.semaphore("dx1") as dx1,
        nc.semaphore("dt0") as dt0,
        nc.semaphore("dt1") as dt1,
        nc.semaphore("ae") as ae,
        nc.semaphore("dv") as dv,
        nc.semaphore("dout") as dout,
    ):
        def xs(i):  # x tile slot (2D [P, fd] AP)
            off = (i % 2) * fd
            return x_sb[:, off : off + fd]

        def ts(i):
            off = (i % 2) * fd
            return t_sb[:, off : off + fd]

        def es(i):
            off = (i % 2) * fd
            return e_sb[:, off : off + fd]

        def v3(ap):
            return ap.rearrange("p (n w) -> p n w", w=W)

        # per-slot DMA-completion sems: two in-flight DMAs on one sem can
        # complete out of order (16 SDMA engines drain both concurrently),
        # so a shared counting sem would not identify which DMA landed
        dxs = [dx0, dx1]
        dts = [dt0, dt1]

        # NEFF re-execution starts with whatever semaphore values the last
        # run left behind; clear them (same preamble Bass emits when
        # target_bir_lowering=True), then barrier so no engine races ahead
        for sem_range in compact_to_ranges(
            [sm for sm in nc._kernel_sem_range if sm not in nc.barrier_sems]
        ):
            nc.gpsimd.dma_reset(sem_range)
            nc.gpsimd.sem_clear(sem_range)
        nc._nrt_pseudo_barrier()

        with nc.Block() as block:

            @block.sync
            def _(sp: bass.BassEngine):
                for i in range(nt):
                    rs = slice(i * t_rows, (i + 1) * t_rows)
                    if i >= 2:
                        # x slot reuse: exp(i-2) has read it (implies DMA done)
                        sp.wait_ge(ae, i - 1)
                    sp.dma_start(out=v3(xs(i)), in_=pred_v[:, rs, :]).then_inc(
                        dxs[i % 2], 16
                    )
                    if i >= 2:
                        # t slot reuse: reduce_u(i-2) done (3rd DVE op of tile)
                        sp.wait_ge(dv, 3 * (i - 2) + 3)
                    sp.dma_start(out=v3(ts(i)), in_=targ_v[:, rs, :]).then_inc(
                        dts[i % 2], 16
                    )
                # final row-math produces out_t after dv = 3*nt + 3
                sp.wait_ge(dv, 3 * nt + 3)
                sp.dma_start(out=part[:], in_=out_t[:]).then_inc(dout, 16)
                sp.wait_ge(dout, 16)

            @block.scalar
            def _(act: bass.BassScalarEngine):
                for i in range(nt):
                    act.wait_ge(dxs[i % 2], 16 * (i // 2 + 1))
                    if i >= 2:
                        # e slot reuse: mult(i-2) has read it (2nd DVE op)
                        act.wait_ge(dv, 3 * (i - 2) + 2)
                    act.activation(out=es(i), in_=xs(i), func=AF.Exp).then_inc(
                        ae, 1
                    )
                # finale: a = ln(u) - ln(s), p = exp(a), g = exp(1-p)
                act.wait_ge(dv, 3 * nt)
                act.activation(out=s_all[:], in_=s_all[:], func=AF.Ln).then_inc(
                    ae, 1
                )
                act.activation(out=z_all[:], in_=z_all[:], func=AF.Ln).then_inc(
                    ae, 1
                )
                act.wait_ge(dv, 3 * nt + 1)  # a ready in z_all
                act.activation(out=g_all[:], in_=z_all[:], func=AF.Exp).then_inc(
                    ae, 1
                )
                act.wait_ge(ae, nt + 3)  # own exp landed (same-engine RAW)
                act.activation(
                    out=g_all[:], in_=g_all[:], func=AF.Exp, scale=-1.0, bias=1.0
                ).then_inc(ae, 1)

            @block.vector
            def _(ve: bass.BassVectorEngine):
                for i in range(nt):
                    rs = slice(i * t_rows, (i + 1) * t_rows)
                    ve.wait_ge(ae, i + 1)
                    ve.tensor_reduce(
                        out=s_all[:, rs], in_=v3(es(i)), axis=AX.X, op=ALU.add
                    ).then_inc(dv, 1)
                    ve.wait_ge(dts[i % 2], 16 * (i // 2 + 1))
                    ve.scalar_tensor_tensor(
                        out=ts(i), in0=ts(i), scalar=1.0, in1=es(i),
                        op0=ALU.mult, op1=ALU.mult,
                    ).then_inc(dv, 1)
                    # same-engine RAW on t: DVE writes land asynchronously,
                    # program order alone is not enough
                    ve.wait_ge(dv, 3 * i + 2)
                    ve.tensor_reduce(
                        out=z_all[:, rs], in_=v3(ts(i)), axis=AX.X, op=ALU.add
                    ).then_inc(dv, 1)
                # finale
                ve.wait_ge(ae, nt + 2)  # both Ln done
                ve.tensor_tensor(
                    out=z_all[:], in0=z_all[:], in1=s_all[:], op=ALU.subtract
                ).then_inc(dv, 1)
                ve.wait_ge(ae, nt + 4)  # g ready
                ve.wait_ge(dv, 3 * nt + 1)  # own subtract (a) landed
                ve.scalar_tensor_tensor(
                    out=g_all[:], in0=g_all[:], scalar=1.0, in1=z_all[:],
                    op0=ALU.subtract, op1=ALU.mult,
                ).then_inc(dv, 1)
                ve.wait_ge(dv, 3 * nt + 2)  # own q landed
                ve.tensor_reduce(
                    out=out_t[:], in_=g_all[:], axis=AX.X, op=ALU.add
                ).then_inc(dv, 1)

    nc.finalize()
    return nc


def _get_nc():
    key = (B_LOC, 128)
    if key not in _CACHE:
        _CACHE[key] = _build()
    return _CACHE[key]


def kernel(predict, target, penalty_matrix):
    from concourse.bass_utils import run_bass_kernel_spmd

    predict = np.ascontiguousarray(np.asarray(predict, dtype=np.float32))
    target = np.ascontiguousarray(np.asarray(target, dtype=np.float32))

    nc = _get_nc()
    in_maps = [
        {
            "predict": predict[i * B_LOC : (i + 1) * B_LOC],
            "target": target[i * B_LOC : (i + 1) * B_LOC],
        }
        for i in range(NCORES)
    ]
    res = run_bass_kernel_spmd(nc, in_maps, list(range(NCORES)))
    total = sum(
        np.asarray(r["partial"], dtype=np.float64).sum() for r in res.results
    )
    return np.float32(-total / B)
